# revision 11
# baseline (speedup 1.0000x reference)
"""Atomwise (segment_reduce) Trainium2 kernel.

y[m] = sum_{atoms i in molecule m} (x[i] . W[0] + b[0]),  m in [0, 100000)

Strategy (8 NeuronCores, SPMD, no collectives needed):
  - Host cuts the (sorted) atom axis at molecule boundaries into 8 nearly
    equal shards -> each core owns a disjoint, contiguous molecule range.
  - Per core, molecules are processed in chunks of 128 (the PSUM partition
    dim).  The host materializes, per chunk, a fixed-size window of A_max
    atom rows in bf16 with a ones-column appended (counts column for the
    bias term), so every core runs the SAME static graph and each DMA
    descriptor moves one contiguous ~5.7KB run per partition.
  - Atom->partition mapping is p-major: partition p holds the chunk's
    atoms [p*NB, (p+1)*NB) as NB row-groups of 129 values (128 feat + 1).
  - Device per chunk:
      * one DMA of the [128, NB*129] bf16 tile
      * VectorE builds one-hot H[p, j, m] = (lidx[p, j] == m) in bf16 with
        a single is_equal over the whole chunk (lidx broadcast vs iota)
      * TensorE accumulates S_aug[128 mols, 129] = sum_j H_j^T @ Xaug_j in
        PSUM (segment-sum of atom feature rows + per-molecule counts)
      * VectorE: y_all[m, c] = sum_f S_aug[m, f] * w0aug[f]  where
        w0aug = [W[0,:], b[0]]  (scalar_tensor_tensor with accum_out)
  - One output DMA of y_all [128, NCH] at the end; host un-permutes and
    stitches the 8 disjoint per-core molecule ranges into y[100000].
"""

import numpy as np
import ml_dtypes

N_ATOMS = 2_000_000
N_IN = 128
N_MOL = 100_000
NCORES = 8
P = 128
NFA = N_IN + 1  # 128 features + 1 counts column

_graph_cache: dict = {}


def _build_graph(NCH: int, NB: int):
    import concourse.mybir as mybir
    from concourse import bacc
    from concourse.tile import TileContext

    f32 = mybir.dt.float32
    bf16 = mybir.dt.bfloat16
    A_max = NB * P
    QC = 4  # chunks loaded per DMA (bigger per-partition contiguous runs)
    NQ = NCH // QC
    IOTA_OFF = 0
    LIDX_OFF = NB * P
    W0_OFF = LIDX_OFF + NCH * NB
    W0_OFF += W0_OFF % 2
    CW = W0_OFF + 2 * NFA

    nc = bacc.Bacc()
    xw = nc.dram_tensor("xw", [NCH * A_max, NFA], bf16, kind="ExternalInput")
    cst = nc.dram_tensor("cst", [P, CW], bf16, kind="ExternalInput")
    out = nc.dram_tensor("out", [P * NCH], f32, kind="ExternalOutput")

    # row (q, p, j4) -> partition p, free (j4*NFA + f): per-partition
    # contiguous QC*NB*NFA*2 bytes in DRAM per quad of chunks
    xw_r = xw.rearrange("(q p j) f -> q p (j f)", p=P, j=QC * NB)
    out_r = out.rearrange("(p c) -> p c", c=NCH)

    with TileContext(nc) as tc:
        with tc.tile_pool(name="const", bufs=1) as cpool, \
             tc.tile_pool(name="xbp", bufs=3) as xbpool, \
             tc.tile_pool(name="hp", bufs=3) as hpool, \
             tc.tile_pool(name="lwp", bufs=3) as lwpool, \
             tc.tile_pool(name="ep", bufs=2) as epool, \
             tc.tile_pool(name="pp", bufs=4, space="PSUM") as pspool:
            cst_t = cpool.tile([P, CW], bf16)
            nc.sync.dma_start(cst_t[:], cst[:, :])
            iota_w = cst_t[:, IOTA_OFF:IOTA_OFF + NB * P]
            w0_t = cst_t[:, W0_OFF:W0_OFF + 2 * NFA].bitcast(f32)
            y_all = cpool.tile([P, NCH], f32)

            for q in range(NQ):
                xq = xbpool.tile([P, QC * NB * NFA], bf16, tag="xq")
                nc.sync.dma_start(xq[:], xw_r[q])
                for k in range(QC):
                    c = q * QC + k
                    xb = xq[:, k * NB * NFA:(k + 1) * NB * NFA]
                    ht = hpool.tile([P, NB * P], bf16, tag="h")
                    nc.vector.tensor_tensor(
                        out=ht[:].rearrange("p (j f) -> p j f", j=NB),
                        in0=iota_w.rearrange("p (j f) -> p j f", j=NB),
                        in1=cst_t[:, LIDX_OFF + c * NB:LIDX_OFF + (c + 1) * NB
                                  ].to_broadcast([P, NB, P]),
                        op=mybir.AluOpType.is_equal,
                    )
                    ps = pspool.tile([P, NFA], f32, tag="ps")
                    for j in range(NB):
                        nc.tensor.matmul(
                            ps[:],
                            lhsT=ht[:, j * P:(j + 1) * P],
                            rhs=xb[:, j * NFA:(j + 1) * NFA],
                            start=(j == 0),
                            stop=(j == NB - 1),
                        )
                    prod = epool.tile([P, NFA], f32, tag="prod")
                    nc.vector.scalar_tensor_tensor(
                        out=prod[:],
                        in0=ps[:],
                        scalar=1.0,
                        in1=w0_t[:],
                        op0=mybir.AluOpType.mult,
                        op1=mybir.AluOpType.mult,
                        accum_out=y_all[:, c:c + 1],
                    )
            nc.sync.dma_start(out_r[:, :], y_all[:])
    nc.finalize()
    return nc


def _prep(inputs):
    x = np.ascontiguousarray(np.asarray(inputs["scalar_representation"], dtype=np.float32))
    idx = np.asarray(inputs["idx_m"]).astype(np.int64)
    W = np.asarray(inputs["W"], dtype=np.float32)
    b = np.asarray(inputs["b"], dtype=np.float32)
    n = x.shape[0]

    # mol_start[m] = first atom index belonging to molecule m (m in 0..N_MOL)
    mol_start = np.searchsorted(idx, np.arange(N_MOL + 1), side="left")

    # Cut cores at molecule boundaries near equal-atom splits
    targets = (np.arange(NCORES + 1) * n) // NCORES
    mcut = np.searchsorted(mol_start, targets, side="left").astype(np.int64)
    mcut[0], mcut[-1] = 0, N_MOL

    m_counts = np.diff(mcut)
    QC = 4
    NCH = -(-int(np.ceil(m_counts.max() / P)) // QC) * QC

    # Max atoms spanned by any 128-molecule chunk on any core
    span_max = 0
    core_chunks = []  # per core: list of (astart, aend, gm)
    for i in range(NCORES):
        chunks = []
        for c in range(NCH):
            gm = mcut[i] + c * P
            gm_end = min(gm + P, mcut[i + 1])
            if gm >= mcut[i + 1]:
                chunks.append((0, 0, 0))
                continue
            astart = int(mol_start[gm])
            aend = int(mol_start[gm_end])
            chunks.append((astart, aend, int(gm)))
            span_max = max(span_max, aend - astart)
        core_chunks.append(chunks)
    NB = max(1, int(np.ceil(span_max / P)))
    A_max = NB * P

    IOTA_OFF = 0
    LIDX_OFF = NB * P
    W0_OFF = LIDX_OFF + NCH * NB
    W0_OFF += W0_OFF % 2
    CW = W0_OFF + 2 * NFA
    iota_row = np.tile(np.arange(P, dtype=np.float32), NB).astype(ml_dtypes.bfloat16)
    w0aug_row = np.concatenate([W[0], b[0:1]]).astype(np.float32).view(ml_dtypes.bfloat16)

    in_maps = []
    for i in range(NCORES):
        xw_i = np.zeros((NCH * A_max, NFA), dtype=ml_dtypes.bfloat16)
        lidx_flat = np.full(NCH * A_max, -1.0, dtype=np.float32)
        for c, (astart, aend, gm) in enumerate(core_chunks[i]):
            spn = aend - astart
            if spn <= 0:
                continue
            xw_i[c * A_max:c * A_max + spn, 0:N_IN] = x[astart:aend]
            xw_i[c * A_max:c * A_max + spn, N_IN] = 1.0
            lidx_flat[c * A_max:c * A_max + spn] = idx[astart:aend] - gm
        # lidx layout [P, NCH*NB]: col (c*NB + j), row p -> atom (c, p*NB + j)
        lidx_t = lidx_flat.reshape(NCH, P, NB).transpose(1, 0, 2).reshape(
            P, NCH * NB).astype(ml_dtypes.bfloat16)
        # quad-major xw: row order (q, p, k, j) so each partition gets one
        # contiguous QC*NB*NFA*2-byte run per quad DMA
        xw_i = np.ascontiguousarray(
            xw_i.reshape(NCH // QC, QC, P, NB, NFA)
                .transpose(0, 2, 1, 3, 4)
                .reshape(NCH * A_max, NFA))
        cst = np.zeros((P, CW), dtype=ml_dtypes.bfloat16)
        cst[:, IOTA_OFF:IOTA_OFF + NB * P] = iota_row[None, :]
        cst[:, LIDX_OFF:LIDX_OFF + NCH * NB] = lidx_t
        cst[:, W0_OFF:W0_OFF + 2 * NFA] = w0aug_row[None, :]
        in_maps.append({"xw": xw_i, "cst": np.ascontiguousarray(cst)})
    return in_maps, mcut, m_counts, NCH, NB


def _run(inputs, trace=False):
    from concourse import bass_utils

    in_maps, mcut, m_counts, NCH, NB = _prep(inputs)
    key = (NCH, NB)
    if key not in _graph_cache:
        _graph_cache[key] = _build_graph(NCH, NB)
    nc = _graph_cache[key]

    res = bass_utils.run_bass_kernel_spmd(
        nc, in_maps, core_ids=list(range(NCORES)), trace=trace
    )
    y = np.zeros(N_MOL, dtype=np.float32)
    for i in range(NCORES):
        mc = int(m_counts[i])
        arr = res.results[i]["out"].reshape(P, NCH).T.ravel()
        y[mcut[i]:mcut[i] + mc] = arr[:mc]
    return y, res


def kernel(**inputs) -> np.ndarray:
    y, _ = _run(inputs, trace=False)
    return y


# revision 13
# speedup vs baseline: 1.4070x; 1.4070x over previous
"""Atomwise (segment_reduce) Trainium2 kernel.

y[m] = sum_{atoms i in molecule m} (x[i] . W[0] + b[0]),  m in [0, 100000)

Strategy (8 NeuronCores, SPMD, no collectives needed):
  - Host cuts the (sorted) atom axis at molecule boundaries into 8 nearly
    equal shards -> each core owns a disjoint, contiguous molecule range.
  - Per core, molecules are processed in chunks of 128 (the PSUM partition
    dim).  The host materializes, per chunk, a fixed-size window of A_max
    atom rows in bf16 with a ones-column appended (counts column for the
    bias term), so every core runs the SAME static graph and each DMA
    descriptor moves one contiguous ~5.7KB run per partition.
  - Atom->partition mapping is p-major: partition p holds the chunk's
    atoms [p*NB, (p+1)*NB) as NB row-groups of 129 values (128 feat + 1).
  - Device per chunk:
      * one DMA of the [128, NB*129] bf16 tile
      * VectorE builds one-hot H[p, j, m] = (lidx[p, j] == m) in bf16 with
        a single is_equal over the whole chunk (lidx broadcast vs iota)
      * TensorE accumulates S_aug[128 mols, 129] = sum_j H_j^T @ Xaug_j in
        PSUM (segment-sum of atom feature rows + per-molecule counts)
      * VectorE: y_all[m, c] = sum_f S_aug[m, f] * w0aug[f]  where
        w0aug = [W[0,:], b[0]]  (scalar_tensor_tensor with accum_out)
  - One output DMA of y_all [128, NCH] at the end; host un-permutes and
    stitches the 8 disjoint per-core molecule ranges into y[100000].
"""

import numpy as np
import ml_dtypes

N_ATOMS = 2_000_000
N_IN = 128
N_MOL = 100_000
NCORES = 8
P = 128
NFA = N_IN + 1  # 128 features + 1 counts column

_graph_cache: dict = {}


def _chunk_groups(NCH):
    groups = []
    c = 0
    for sz in (8, 4, 2, 1):
        while NCH - c >= sz:
            groups.append((c, sz))
            c += sz
    return groups


def _build_graph(NCH: int, NB: int):
    import concourse.mybir as mybir
    from concourse import bacc
    from concourse.tile import TileContext

    f32 = mybir.dt.float32
    bf16 = mybir.dt.bfloat16
    A_max = NB * P
    IOTA_OFF = 0
    LIDX_OFF = NB * P
    W0_OFF = LIDX_OFF + NCH * NB
    W0_OFF += W0_OFF % 2
    CW = W0_OFF + 2 * NFA

    nc = bacc.Bacc()
    xw = nc.dram_tensor("xw", [NCH * A_max, NFA], bf16, kind="ExternalInput")
    cst = nc.dram_tensor("cst", [P, CW], bf16, kind="ExternalInput")
    out = nc.dram_tensor("out", [P * NCH], f32, kind="ExternalOutput")

    out_r = out.rearrange("(p c) -> p c", c=NCH)
    groups = _chunk_groups(NCH)

    with TileContext(nc) as tc:
        with tc.tile_pool(name="const", bufs=1) as cpool, \
             tc.tile_pool(name="xbp", bufs=3) as xbpool, \
             tc.tile_pool(name="hp", bufs=3) as hpool, \
             tc.tile_pool(name="lwp", bufs=3) as lwpool, \
             tc.tile_pool(name="ep", bufs=2) as epool, \
             tc.tile_pool(name="pp", bufs=4, space="PSUM") as pspool:
            cst_t = cpool.tile([P, CW], bf16)
            nc.sync.dma_start(cst_t[:], cst[:, :])
            iota_w = cst_t[:, IOTA_OFF:IOTA_OFF + NB * P]
            w0_t = cst_t[:, W0_OFF:W0_OFF + 2 * NFA].bitcast(f32)
            y_all = cpool.tile([P, NCH], f32)

            for gstart, gc in groups:
                # rows for this group: [gstart*A_max, (gstart+gc)*A_max), laid
                # out (p, k, j): per-partition contiguous gc*NB*NFA elements
                xq = xbpool.tile([P, gc * NB * NFA], bf16, tag="xq")
                nc.sync.dma_start(
                    xq[:],
                    xw[gstart * A_max:(gstart + gc) * A_max, :].rearrange(
                        "(p j) f -> p (j f)", p=P),
                )
                for k in range(gc):
                    c = gstart + k
                    xb = xq[:, k * NB * NFA:(k + 1) * NB * NFA]
                    lw = lwpool.tile([P, NB * P], bf16, tag="lw")
                    exp_eng = nc.vector if (c % 6 == 5) else nc.scalar
                    if exp_eng is nc.scalar:
                        nc.scalar.activation(
                            lw[:].rearrange("p (j f) -> p j f", j=NB),
                            cst_t[:, LIDX_OFF + c * NB:LIDX_OFF + (c + 1) * NB
                                  ].to_broadcast([P, NB, P]),
                            mybir.ActivationFunctionType.Copy,
                        )
                    else:
                        nc.vector.tensor_copy(
                            lw[:].rearrange("p (j f) -> p j f", j=NB),
                            cst_t[:, LIDX_OFF + c * NB:LIDX_OFF + (c + 1) * NB
                                  ].to_broadcast([P, NB, P]),
                        )
                    ht = hpool.tile([P, NB * P], bf16, tag="h")
                    nc.vector.tensor_tensor(
                        out=ht[:],
                        in0=lw[:],
                        in1=iota_w,
                        op=mybir.AluOpType.is_equal,
                    )
                    ps = pspool.tile([P, NFA], f32, tag="ps")
                    for j in range(NB):
                        nc.tensor.matmul(
                            ps[:],
                            lhsT=ht[:, j * P:(j + 1) * P],
                            rhs=xb[:, j * NFA:(j + 1) * NFA],
                            start=(j == 0),
                            stop=(j == NB - 1),
                        )
                    prod = epool.tile([P, NFA], f32, tag="prod")
                    nc.vector.scalar_tensor_tensor(
                        out=prod[:],
                        in0=ps[:],
                        scalar=1.0,
                        in1=w0_t[:],
                        op0=mybir.AluOpType.mult,
                        op1=mybir.AluOpType.mult,
                        accum_out=y_all[:, c:c + 1],
                    )
            nc.sync.dma_start(out_r[:, :], y_all[:])
    nc.finalize()
    return nc


def _prep(inputs):
    x = np.ascontiguousarray(np.asarray(inputs["scalar_representation"], dtype=np.float32))
    idx = np.asarray(inputs["idx_m"]).astype(np.int64)
    W = np.asarray(inputs["W"], dtype=np.float32)
    b = np.asarray(inputs["b"], dtype=np.float32)
    n = x.shape[0]

    # mol_start[m] = first atom index belonging to molecule m (m in 0..N_MOL)
    mol_start = np.searchsorted(idx, np.arange(N_MOL + 1), side="left")

    # Cut cores at molecule boundaries near equal-atom splits
    targets = (np.arange(NCORES + 1) * n) // NCORES
    mcut = np.searchsorted(mol_start, targets, side="left").astype(np.int64)
    mcut[0], mcut[-1] = 0, N_MOL

    m_counts = np.diff(mcut)
    NCH = int(np.ceil(m_counts.max() / P))

    # Max atoms spanned by any 128-molecule chunk on any core
    span_max = 0
    core_chunks = []  # per core: list of (astart, aend, gm)
    for i in range(NCORES):
        chunks = []
        for c in range(NCH):
            gm = mcut[i] + c * P
            gm_end = min(gm + P, mcut[i + 1])
            if gm >= mcut[i + 1]:
                chunks.append((0, 0, 0))
                continue
            astart = int(mol_start[gm])
            aend = int(mol_start[gm_end])
            chunks.append((astart, aend, int(gm)))
            span_max = max(span_max, aend - astart)
        core_chunks.append(chunks)
    NB = max(1, int(np.ceil(span_max / P)))
    A_max = NB * P

    IOTA_OFF = 0
    LIDX_OFF = NB * P
    W0_OFF = LIDX_OFF + NCH * NB
    W0_OFF += W0_OFF % 2
    CW = W0_OFF + 2 * NFA
    iota_row = np.tile(np.arange(P, dtype=np.float32), NB).astype(ml_dtypes.bfloat16)
    w0aug_row = np.concatenate([W[0], b[0:1]]).astype(np.float32).view(ml_dtypes.bfloat16)

    in_maps = []
    for i in range(NCORES):
        xw_i = np.zeros((NCH * A_max, NFA), dtype=ml_dtypes.bfloat16)
        lidx_flat = np.full(NCH * A_max, -1.0, dtype=np.float32)
        for c, (astart, aend, gm) in enumerate(core_chunks[i]):
            spn = aend - astart
            if spn <= 0:
                continue
            xw_i[c * A_max:c * A_max + spn, 0:N_IN] = x[astart:aend]
            xw_i[c * A_max:c * A_max + spn, N_IN] = 1.0
            lidx_flat[c * A_max:c * A_max + spn] = idx[astart:aend] - gm
        # lidx layout [P, NCH*NB]: col (c*NB + j), row p -> atom (c, p*NB + j)
        lidx_t = lidx_flat.reshape(NCH, P, NB).transpose(1, 0, 2).reshape(
            P, NCH * NB).astype(ml_dtypes.bfloat16)
        # group-major xw: within each DMA group, row order (p, k, j) so each
        # partition gets one contiguous gc*NB*NFA*2-byte run per group DMA
        parts = []
        for gstart, gc in _chunk_groups(NCH):
            blk = xw_i[gstart * A_max:(gstart + gc) * A_max]
            parts.append(np.ascontiguousarray(
                blk.reshape(gc, P, NB, NFA).transpose(1, 0, 2, 3)
                   .reshape(gc * A_max, NFA)))
        xw_i = np.concatenate(parts, axis=0)
        cst = np.zeros((P, CW), dtype=ml_dtypes.bfloat16)
        cst[:, IOTA_OFF:IOTA_OFF + NB * P] = iota_row[None, :]
        cst[:, LIDX_OFF:LIDX_OFF + NCH * NB] = lidx_t
        cst[:, W0_OFF:W0_OFF + 2 * NFA] = w0aug_row[None, :]
        in_maps.append({"xw": xw_i, "cst": np.ascontiguousarray(cst)})
    return in_maps, mcut, m_counts, NCH, NB


def _run(inputs, trace=False):
    from concourse import bass_utils

    in_maps, mcut, m_counts, NCH, NB = _prep(inputs)
    key = (NCH, NB)
    if key not in _graph_cache:
        _graph_cache[key] = _build_graph(NCH, NB)
    nc = _graph_cache[key]

    res = bass_utils.run_bass_kernel_spmd(
        nc, in_maps, core_ids=list(range(NCORES)), trace=trace
    )
    y = np.zeros(N_MOL, dtype=np.float32)
    for i in range(NCORES):
        mc = int(m_counts[i])
        arr = res.results[i]["out"].reshape(P, NCH).T.ravel()
        y[mcut[i]:mcut[i] + mc] = arr[:mc]
    return y, res


def kernel(**inputs) -> np.ndarray:
    y, _ = _run(inputs, trace=False)
    return y


# revision 14
# speedup vs baseline: 1.4278x; 1.0148x over previous
"""Atomwise (segment_reduce) Trainium2 kernel.

y[m] = sum_{atoms i in molecule m} (x[i] . W[0] + b[0]),  m in [0, 100000)

8 NeuronCores, SPMD, no collectives: host cuts the (sorted) atom axis at
molecule boundaries into 8 shards; each core owns a disjoint contiguous
molecule range.  Molecules are packed greedily into chunks of up to M=96
consecutive molecules whose atoms fit in A_max = NB*128 rows; the host
materializes per-chunk windows in bf16 (+ ones column for the count/bias
term) in a (partition-major, DMA-group-contiguous) layout.

Device per chunk:
  * DMA (grouped, ~33KB contiguous per partition per group of 8 chunks)
  * lidx expansion (broadcast -> wide) on ScalarE (most chunks) or
    VectorE (a few, to balance), then one VectorE is_equal against a
    tiled iota -> one-hot H [128 atoms, NB, M] in bf16 (2x DVE mode)
  * TensorE accumulates S_aug[M mols, 129] = sum_j H_j^T @ Xaug_j in PSUM
  * VectorE scalar_tensor_tensor: y_all[m, c] = sum_f S_aug[m,f]*w0aug[f]
One output DMA of y_all [M, NCH] at the end; host unpacks chunk ranges.
"""

import numpy as np
import ml_dtypes

N_ATOMS = 2_000_000
N_IN = 128
N_MOL = 100_000
NCORES = 8
P = 128
NFA = N_IN + 1  # 128 features + 1 counts column
M = 96          # molecules per chunk (PSUM partition dim of S_aug)
NB = 16         # 128-atom blocks per chunk window (A_max = 2048)
DVE_EXPAND_EVERY = 9  # 1 of every k chunks expands lidx on VectorE

_graph_cache: dict = {}


def _chunk_groups(NCH):
    groups, c = [], 0
    for sz in (8, 4, 2, 1):
        while NCH - c >= sz:
            groups.append((c, sz))
            c += sz
    return groups


def _build_graph(NCH: int):
    import concourse.mybir as mybir
    from concourse import bacc
    from concourse.tile import TileContext

    f32 = mybir.dt.float32
    bf16 = mybir.dt.bfloat16
    A_max = NB * P
    IOTA_OFF = 0
    LIDX_OFF = NB * M
    W0_OFF = LIDX_OFF + NCH * NB
    W0_OFF += W0_OFF % 2
    CW = W0_OFF + 2 * NFA

    nc = bacc.Bacc()
    xw = nc.dram_tensor("xw", [NCH * A_max, NFA], bf16, kind="ExternalInput")
    cst = nc.dram_tensor("cst", [P, CW], bf16, kind="ExternalInput")
    out = nc.dram_tensor("out", [M * NCH], f32, kind="ExternalOutput")
    out_r = out.rearrange("(p c) -> p c", c=NCH)
    groups = _chunk_groups(NCH)

    with TileContext(nc) as tc:
        with tc.tile_pool(name="const", bufs=1) as cpool, \
             tc.tile_pool(name="xbp", bufs=3) as xbpool, \
             tc.tile_pool(name="hp", bufs=3) as hpool, \
             tc.tile_pool(name="lwp", bufs=3) as lwpool, \
             tc.tile_pool(name="ep", bufs=2) as epool, \
             tc.tile_pool(name="pp", bufs=4, space="PSUM") as pspool:
            cst_t = cpool.tile([P, CW], bf16)
            nc.sync.dma_start(cst_t[:], cst[:, :])
            iota_w = cst_t[:, IOTA_OFF:IOTA_OFF + NB * M]
            w0_t = cst_t[:, W0_OFF:W0_OFF + 2 * NFA].bitcast(f32)
            y_all = cpool.tile([P, NCH], f32)

            for gstart, gc in groups:
                xq = xbpool.tile([P, gc * NB * NFA], bf16, tag="xq")
                nc.sync.dma_start(
                    xq[:],
                    xw[gstart * A_max:(gstart + gc) * A_max, :].rearrange(
                        "(p j) f -> p (j f)", p=P),
                )
                for k in range(gc):
                    c = gstart + k
                    xb = xq[:, k * NB * NFA:(k + 1) * NB * NFA]
                    lw = lwpool.tile([P, NB * M], bf16, tag="lw")
                    lsrc = cst_t[:, LIDX_OFF + c * NB:LIDX_OFF + (c + 1) * NB
                                 ].to_broadcast([P, NB, M])
                    if c % DVE_EXPAND_EVERY == DVE_EXPAND_EVERY - 1:
                        nc.vector.tensor_copy(
                            lw[:].rearrange("p (j f) -> p j f", j=NB), lsrc)
                    else:
                        nc.scalar.activation(
                            lw[:].rearrange("p (j f) -> p j f", j=NB), lsrc,
                            mybir.ActivationFunctionType.Copy)
                    ht = hpool.tile([P, NB * M], bf16, tag="h")
                    nc.vector.tensor_tensor(
                        out=ht[:], in0=lw[:], in1=iota_w,
                        op=mybir.AluOpType.is_equal)
                    ps = pspool.tile([M, NFA], f32, tag="ps")
                    for j in range(NB):
                        nc.tensor.matmul(
                            ps[:],
                            lhsT=ht[:, j * M:(j + 1) * M],
                            rhs=xb[:, j * NFA:(j + 1) * NFA],
                            start=(j == 0),
                            stop=(j == NB - 1),
                        )
                    prod = epool.tile([M, NFA], f32, tag="prod")
                    nc.vector.scalar_tensor_tensor(
                        out=prod[:],
                        in0=ps[:],
                        scalar=1.0,
                        in1=w0_t[0:M, :],
                        op0=mybir.AluOpType.mult,
                        op1=mybir.AluOpType.mult,
                        accum_out=y_all[0:M, c:c + 1],
                    )
            nc.sync.dma_start(out_r[:, :], y_all[0:M, :])
    nc.finalize()
    return nc


def _prep(inputs):
    x = np.ascontiguousarray(np.asarray(inputs["scalar_representation"], dtype=np.float32))
    idx = np.asarray(inputs["idx_m"]).astype(np.int64)
    W = np.asarray(inputs["W"], dtype=np.float32)
    b = np.asarray(inputs["b"], dtype=np.float32)
    n = x.shape[0]
    A_max = NB * P

    mol_start = np.searchsorted(idx, np.arange(N_MOL + 1), side="left")

    targets = (np.arange(NCORES + 1) * n) // NCORES
    mcut = np.searchsorted(mol_start, targets, side="left").astype(np.int64)
    mcut[0], mcut[-1] = 0, N_MOL

    # Greedy chunking per core: up to M consecutive molecules per chunk,
    # atoms must fit in A_max rows.
    core_chunks = []  # per core: list of (astart, aend, gm, nmols)
    for i in range(NCORES):
        chunks = []
        gm = int(mcut[i])
        gend = int(mcut[i + 1])
        while gm < gend:
            hi = min(gm + M, gend)
            # largest nm <= hi-gm with span <= A_max
            while mol_start[gm + (hi - gm)] - mol_start[gm] > A_max:
                hi -= 8
            nm = hi - gm
            assert nm > 0
            chunks.append((int(mol_start[gm]), int(mol_start[hi]), gm, nm))
            gm = hi
        core_chunks.append(chunks)
    NCH = max(len(ch) for ch in core_chunks)

    IOTA_OFF = 0
    LIDX_OFF = NB * M
    W0_OFF = LIDX_OFF + NCH * NB
    W0_OFF += W0_OFF % 2
    CW = W0_OFF + 2 * NFA
    iota_row = np.tile(np.arange(M, dtype=np.float32), NB).astype(ml_dtypes.bfloat16)
    w0aug_row = np.concatenate([W[0], b[0:1]]).astype(np.float32).view(ml_dtypes.bfloat16)

    in_maps = []
    for i in range(NCORES):
        chunks = core_chunks[i]
        xw_i = np.zeros((NCH * A_max, NFA), dtype=ml_dtypes.bfloat16)
        lidx_flat = np.full(NCH * A_max, -1.0, dtype=np.float32)
        for c, (astart, aend, gm, nm) in enumerate(chunks):
            spn = aend - astart
            if spn <= 0:
                continue
            xw_i[c * A_max:c * A_max + spn, 0:N_IN] = x[astart:aend]
            xw_i[c * A_max:c * A_max + spn, N_IN] = 1.0
            lidx_flat[c * A_max:c * A_max + spn] = idx[astart:aend] - gm
        lidx_t = lidx_flat.reshape(NCH, P, NB).transpose(1, 0, 2).reshape(
            P, NCH * NB).astype(ml_dtypes.bfloat16)
        # group-major xw: per DMA group, row order (p, k, j)
        parts = []
        for gstart, gc in _chunk_groups(NCH):
            blk = xw_i[gstart * A_max:(gstart + gc) * A_max]
            parts.append(np.ascontiguousarray(
                blk.reshape(gc, P, NB, NFA).transpose(1, 0, 2, 3)
                   .reshape(gc * A_max, NFA)))
        xw_i = np.concatenate(parts, axis=0)
        cst = np.zeros((P, CW), dtype=ml_dtypes.bfloat16)
        cst[:, IOTA_OFF:IOTA_OFF + NB * M] = iota_row[None, :]
        cst[:, LIDX_OFF:LIDX_OFF + NCH * NB] = lidx_t
        cst[:, W0_OFF:W0_OFF + 2 * NFA] = w0aug_row[None, :]
        in_maps.append({"xw": xw_i, "cst": np.ascontiguousarray(cst)})
    return in_maps, core_chunks, NCH


def _run(inputs, trace=False):
    from concourse import bass_utils

    in_maps, core_chunks, NCH = _prep(inputs)
    key = (NCH,)
    if key not in _graph_cache:
        _graph_cache[key] = _build_graph(NCH)
    nc = _graph_cache[key]

    res = bass_utils.run_bass_kernel_spmd(
        nc, in_maps, core_ids=list(range(NCORES)), trace=trace
    )
    y = np.zeros(N_MOL, dtype=np.float32)
    for i in range(NCORES):
        arr = res.results[i]["out"].reshape(M, NCH)
        for c, (astart, aend, gm, nm) in enumerate(core_chunks[i]):
            y[gm:gm + nm] = arr[0:nm, c]
    return y, res


def kernel(**inputs) -> np.ndarray:
    y, _ = _run(inputs, trace=False)
    return y


# revision 15
# speedup vs baseline: 1.5821x; 1.1081x over previous
"""Atomwise (segment_reduce) Trainium2 kernel.

y[m] = sum_{atoms i in molecule m} (x[i] . W[0] + b[0]),  m in [0, 100000)

8 NeuronCores, SPMD, no collectives: host cuts the (sorted) atom axis at
molecule boundaries into 8 shards; each core owns a disjoint contiguous
molecule range.  Molecules are packed greedily into chunks of up to M=96
consecutive molecules whose atoms fit in A_max = NB*128 rows; the host
materializes per-chunk windows in bf16 (+ ones column for the count/bias
term) in a (partition-major, DMA-group-contiguous) layout.

Device pipeline:
  * grouped DMA (~31KB contiguous per partition per group of 8 chunks)
  * lidx expansion (broadcast -> wide) batched 4 chunks per op, mostly on
    ScalarE with some batches on VectorE for load balance
  * VectorE is_equal vs tiled iota, batched 2 chunks per op -> one-hot
    H [128 atoms, NB, M] bf16 (2x DVE mode)
  * TensorE accumulates S_aug[M mols, 129] = sum_j H_j^T @ Xaug_j in PSUM
  * VectorE scalar_tensor_tensor: y_all[m, c] = sum_f S_aug[m,f]*w0aug[f]
One output DMA of y_all [M, NCH] at the end; host unpacks chunk ranges.
"""

import numpy as np
import ml_dtypes

N_ATOMS = 2_000_000
N_IN = 128
N_MOL = 100_000
NCORES = 8
P = 128
NFA = N_IN + 1  # 128 features + 1 counts column
M = 96          # max molecules per chunk (PSUM partition dim of S_aug)
NB = 15         # 128-atom blocks per chunk window (A_max = 1920)
BEX = 4         # chunks per expansion op
BTT = 2         # chunks per is_equal op
DVE_BATCH_EVERY = 10  # every k-th expansion batch runs on VectorE

_graph_cache: dict = {}


def _chunk_groups(NCH):
    groups, c = [], 0
    for sz in (8, 4, 2, 1):
        while NCH - c >= sz:
            groups.append((c, sz))
            c += sz
    return groups


def _build_graph(NCH: int):
    import concourse.mybir as mybir
    from concourse import bacc
    from concourse.tile import TileContext

    f32 = mybir.dt.float32
    bf16 = mybir.dt.bfloat16
    A_max = NB * P
    IOTA_OFF = 0
    LIDX_OFF = BTT * NB * M
    W0_OFF = LIDX_OFF + NCH * NB
    W0_OFF += W0_OFF % 2
    CW = W0_OFF + 2 * NFA

    nc = bacc.Bacc()
    xw = nc.dram_tensor("xw", [NCH * A_max, NFA], bf16, kind="ExternalInput")
    cst = nc.dram_tensor("cst", [P, CW], bf16, kind="ExternalInput")
    out = nc.dram_tensor("out", [M * NCH], f32, kind="ExternalOutput")
    out_r = out.rearrange("(p c) -> p c", c=NCH)
    groups = _chunk_groups(NCH)

    with TileContext(nc) as tc:
        with tc.tile_pool(name="const", bufs=1) as cpool, \
             tc.tile_pool(name="xbp", bufs=3) as xbpool, \
             tc.tile_pool(name="hp", bufs=3) as hpool, \
             tc.tile_pool(name="lwp", bufs=3) as lwpool, \
             tc.tile_pool(name="ep", bufs=2) as epool, \
             tc.tile_pool(name="pp", bufs=4, space="PSUM") as pspool:
            cst_t = cpool.tile([P, CW], bf16)
            nc.sync.dma_start(cst_t[:], cst[:, :])
            w0_t = cst_t[:, W0_OFF:W0_OFF + 2 * NFA].bitcast(f32)
            y_all = cpool.tile([P, NCH], f32)
            ex_batch = 0

            for gstart, gc in groups:
                xq = xbpool.tile([P, gc * NB * NFA], bf16, tag="xq")
                nc.sync.dma_start(
                    xq[:],
                    xw[gstart * A_max:(gstart + gc) * A_max, :].rearrange(
                        "(p j) f -> p (j f)", p=P),
                )
                b0 = 0
                while b0 < gc:
                    bsz = min(BEX, gc - b0)
                    cb = gstart + b0
                    lw = lwpool.tile([P, BEX * NB * M], bf16, tag="lw")
                    lsrc = cst_t[:, LIDX_OFF + cb * NB:
                                 LIDX_OFF + (cb + bsz) * NB
                                 ].to_broadcast([P, bsz * NB, M])
                    ldst = lw[:, 0:bsz * NB * M].rearrange(
                        "p (j f) -> p j f", j=bsz * NB)
                    if ex_batch % DVE_BATCH_EVERY == DVE_BATCH_EVERY - 1:
                        nc.vector.tensor_copy(ldst, lsrc)
                    else:
                        nc.scalar.activation(
                            ldst, lsrc, mybir.ActivationFunctionType.Copy)
                    ex_batch += 1
                    t0 = 0
                    while t0 < bsz:
                        tsz = min(BTT, bsz - t0)
                        ht = hpool.tile([P, BTT * NB * M], bf16, tag="h")
                        nc.vector.tensor_tensor(
                            out=ht[:, 0:tsz * NB * M],
                            in0=lw[:, (t0) * NB * M:(t0 + tsz) * NB * M],
                            in1=cst_t[:, IOTA_OFF:IOTA_OFF + tsz * NB * M],
                            op=mybir.AluOpType.is_equal)
                        for u in range(tsz):
                            c = cb + t0 + u
                            xb = xq[:, (b0 + t0 + u) * NB * NFA:
                                    (b0 + t0 + u + 1) * NB * NFA]
                            ps = pspool.tile([M, NFA], f32, tag="ps")
                            for j in range(NB):
                                nc.tensor.matmul(
                                    ps[:],
                                    lhsT=ht[:, (u * NB + j) * M:
                                            (u * NB + j + 1) * M],
                                    rhs=xb[:, j * NFA:(j + 1) * NFA],
                                    start=(j == 0),
                                    stop=(j == NB - 1),
                                )
                            prod = epool.tile([M, NFA], f32, tag="prod")
                            nc.vector.scalar_tensor_tensor(
                                out=prod[:],
                                in0=ps[:],
                                scalar=1.0,
                                in1=w0_t[0:M, :],
                                op0=mybir.AluOpType.mult,
                                op1=mybir.AluOpType.mult,
                                accum_out=y_all[0:M, c:c + 1],
                            )
                        t0 += tsz
                    b0 += bsz
            nc.sync.dma_start(out_r[:, :], y_all[0:M, :])
    nc.finalize()
    return nc


def _prep(inputs):
    x = np.ascontiguousarray(np.asarray(inputs["scalar_representation"], dtype=np.float32))
    idx = np.asarray(inputs["idx_m"]).astype(np.int64)
    W = np.asarray(inputs["W"], dtype=np.float32)
    b = np.asarray(inputs["b"], dtype=np.float32)
    n = x.shape[0]
    A_max = NB * P

    mol_start = np.searchsorted(idx, np.arange(N_MOL + 1), side="left")

    targets = (np.arange(NCORES + 1) * n) // NCORES
    mcut = np.searchsorted(mol_start, targets, side="left").astype(np.int64)
    mcut[0], mcut[-1] = 0, N_MOL

    # Greedy chunking per core: up to M consecutive molecules per chunk,
    # atoms must fit in A_max rows (exact via searchsorted).
    core_chunks = []  # per core: list of (astart, aend, gm, nmols)
    for i in range(NCORES):
        chunks = []
        gm = int(mcut[i])
        gend = int(mcut[i + 1])
        while gm < gend:
            hi_atom_lim = int(np.searchsorted(
                mol_start, mol_start[gm] + A_max, side="right")) - 1
            hi = min(gm + M, gend, hi_atom_lim)
            nm = hi - gm
            assert nm > 0
            chunks.append((int(mol_start[gm]), int(mol_start[hi]), gm, nm))
            gm = hi
        core_chunks.append(chunks)
    NCH = max(len(ch) for ch in core_chunks)

    IOTA_OFF = 0
    LIDX_OFF = BTT * NB * M
    W0_OFF = LIDX_OFF + NCH * NB
    W0_OFF += W0_OFF % 2
    CW = W0_OFF + 2 * NFA
    iota_row = np.tile(np.arange(M, dtype=np.float32), BTT * NB).astype(
        ml_dtypes.bfloat16)
    w0aug_row = np.concatenate([W[0], b[0:1]]).astype(np.float32).view(ml_dtypes.bfloat16)

    in_maps = []
    for i in range(NCORES):
        chunks = core_chunks[i]
        xw_i = np.zeros((NCH * A_max, NFA), dtype=ml_dtypes.bfloat16)
        lidx_flat = np.full(NCH * A_max, -1.0, dtype=np.float32)
        for c, (astart, aend, gm, nm) in enumerate(chunks):
            spn = aend - astart
            if spn <= 0:
                continue
            xw_i[c * A_max:c * A_max + spn, 0:N_IN] = x[astart:aend]
            xw_i[c * A_max:c * A_max + spn, N_IN] = 1.0
            lidx_flat[c * A_max:c * A_max + spn] = idx[astart:aend] - gm
        lidx_t = lidx_flat.reshape(NCH, P, NB).transpose(1, 0, 2).reshape(
            P, NCH * NB).astype(ml_dtypes.bfloat16)
        parts = []
        for gstart, gc in _chunk_groups(NCH):
            blk = xw_i[gstart * A_max:(gstart + gc) * A_max]
            parts.append(np.ascontiguousarray(
                blk.reshape(gc, P, NB, NFA).transpose(1, 0, 2, 3)
                   .reshape(gc * A_max, NFA)))
        xw_i = np.concatenate(parts, axis=0)
        cst = np.zeros((P, CW), dtype=ml_dtypes.bfloat16)
        cst[:, IOTA_OFF:IOTA_OFF + BTT * NB * M] = iota_row[None, :]
        cst[:, LIDX_OFF:LIDX_OFF + NCH * NB] = lidx_t
        cst[:, W0_OFF:W0_OFF + 2 * NFA] = w0aug_row[None, :]
        in_maps.append({"xw": xw_i, "cst": np.ascontiguousarray(cst)})
    return in_maps, core_chunks, NCH


def _run(inputs, trace=False):
    from concourse import bass_utils

    in_maps, core_chunks, NCH = _prep(inputs)
    key = (NCH,)
    if key not in _graph_cache:
        _graph_cache[key] = _build_graph(NCH)
    nc = _graph_cache[key]

    res = bass_utils.run_bass_kernel_spmd(
        nc, in_maps, core_ids=list(range(NCORES)), trace=trace
    )
    y = np.zeros(N_MOL, dtype=np.float32)
    for i in range(NCORES):
        arr = res.results[i]["out"].reshape(M, NCH)
        for c, (astart, aend, gm, nm) in enumerate(core_chunks[i]):
            y[gm:gm + nm] = arr[0:nm, c]
    return y, res


def kernel(**inputs) -> np.ndarray:
    y, _ = _run(inputs, trace=False)
    return y


# revision 16
# speedup vs baseline: 1.6947x; 1.0711x over previous
"""Atomwise (segment_reduce) Trainium2 kernel.

y[m] = sum_{atoms i in molecule m} (x[i] . W[0] + b[0]),  m in [0, 100000)

8 NeuronCores, SPMD, no collectives: host cuts the (sorted) atom axis at
molecule boundaries into 8 shards; each core owns a disjoint contiguous
molecule range.  Molecules are packed greedily into chunks of up to M=96
consecutive molecules whose atoms fit in A_max = NB*128 rows; the host
materializes per-chunk windows in bf16 (+ ones column for the count/bias
term) in a (partition-major, DMA-group-contiguous) layout.

Device pipeline:
  * grouped DMA (~31KB contiguous per partition per group of 8 chunks)
  * lidx expansion (broadcast -> wide) batched 4 chunks per op, mostly on
    ScalarE with some batches on VectorE for load balance
  * VectorE is_equal vs tiled iota, batched 2 chunks per op -> one-hot
    H [128 atoms, NB, M] bf16 (2x DVE mode)
  * TensorE accumulates S_aug[M mols, 129] = sum_j H_j^T @ Xaug_j in PSUM
  * VectorE scalar_tensor_tensor: y_all[m, c] = sum_f S_aug[m,f]*w0aug[f]
One output DMA of y_all [M, NCH] at the end; host unpacks chunk ranges.
"""

import numpy as np
import ml_dtypes

N_ATOMS = 2_000_000
N_IN = 128
N_MOL = 100_000
NCORES = 8
P = 128
NFA = N_IN + 1  # 128 features + 1 counts column
M = 102         # max molecules per chunk (PSUM partition dim of S_aug)
NB = 16         # 128-atom blocks per chunk window (A_max = 2048)
BEX = 4         # chunks per expansion op
BTT = 2         # chunks per is_equal op
DVE_BATCH_EVERY = 10  # every k-th expansion batch runs on VectorE

_graph_cache: dict = {}


def _chunk_groups(NCH):
    groups, c = [], 0
    for sz in (8, 4, 2, 1):
        while NCH - c >= sz:
            groups.append((c, sz))
            c += sz
    return groups


def _build_graph(NCH: int):
    import concourse.mybir as mybir
    from concourse import bacc
    from concourse.tile import TileContext

    f32 = mybir.dt.float32
    bf16 = mybir.dt.bfloat16
    A_max = NB * P
    IOTA_OFF = 0
    LIDX_OFF = BTT * NB * M
    W0_OFF = LIDX_OFF + NCH * NB
    W0_OFF += W0_OFF % 2
    CW = W0_OFF + 2 * NFA

    nc = bacc.Bacc()
    xw = nc.dram_tensor("xw", [NCH * A_max, NFA], bf16, kind="ExternalInput")
    cst = nc.dram_tensor("cst", [P, CW], bf16, kind="ExternalInput")
    out = nc.dram_tensor("out", [M * NCH], f32, kind="ExternalOutput")
    out_r = out.rearrange("(p c) -> p c", c=NCH)
    groups = _chunk_groups(NCH)

    with TileContext(nc) as tc:
        with tc.tile_pool(name="const", bufs=1) as cpool, \
             tc.tile_pool(name="xbp", bufs=3) as xbpool, \
             tc.tile_pool(name="hp", bufs=3) as hpool, \
             tc.tile_pool(name="lwp", bufs=3) as lwpool, \
             tc.tile_pool(name="ep", bufs=2) as epool, \
             tc.tile_pool(name="pp", bufs=4, space="PSUM") as pspool:
            cst_t = cpool.tile([P, CW], bf16)
            nc.sync.dma_start(cst_t[:], cst[:, :])
            w0_t = cst_t[:, W0_OFF:W0_OFF + 2 * NFA].bitcast(f32)
            y_all = cpool.tile([P, NCH], f32)
            ex_batch = 0

            for gstart, gc in groups:
                xq = xbpool.tile([P, gc * NB * NFA], bf16, tag="xq")
                nc.sync.dma_start(
                    xq[:],
                    xw[gstart * A_max:(gstart + gc) * A_max, :].rearrange(
                        "(p j) f -> p (j f)", p=P),
                )
                b0 = 0
                while b0 < gc:
                    bsz = min(BEX, gc - b0)
                    cb = gstart + b0
                    lw = lwpool.tile([P, BEX * NB * M], bf16, tag="lw")
                    lsrc = cst_t[:, LIDX_OFF + cb * NB:
                                 LIDX_OFF + (cb + bsz) * NB
                                 ].to_broadcast([P, bsz * NB, M])
                    ldst = lw[:, 0:bsz * NB * M].rearrange(
                        "p (j f) -> p j f", j=bsz * NB)
                    if ex_batch % DVE_BATCH_EVERY == DVE_BATCH_EVERY - 1:
                        nc.vector.tensor_copy(ldst, lsrc)
                    else:
                        nc.scalar.activation(
                            ldst, lsrc, mybir.ActivationFunctionType.Copy)
                    ex_batch += 1
                    t0 = 0
                    while t0 < bsz:
                        tsz = min(BTT, bsz - t0)
                        ht = hpool.tile([P, BTT * NB * M], bf16, tag="h")
                        nc.vector.tensor_tensor(
                            out=ht[:, 0:tsz * NB * M],
                            in0=lw[:, (t0) * NB * M:(t0 + tsz) * NB * M],
                            in1=cst_t[:, IOTA_OFF:IOTA_OFF + tsz * NB * M],
                            op=mybir.AluOpType.is_equal)
                        for u in range(tsz):
                            c = cb + t0 + u
                            xb = xq[:, (b0 + t0 + u) * NB * NFA:
                                    (b0 + t0 + u + 1) * NB * NFA]
                            ps = pspool.tile([M, NFA], f32, tag="ps")
                            for j in range(NB):
                                nc.tensor.matmul(
                                    ps[:],
                                    lhsT=ht[:, (u * NB + j) * M:
                                            (u * NB + j + 1) * M],
                                    rhs=xb[:, j * NFA:(j + 1) * NFA],
                                    start=(j == 0),
                                    stop=(j == NB - 1),
                                )
                            prod = epool.tile([M, NFA], f32, tag="prod")
                            nc.vector.scalar_tensor_tensor(
                                out=prod[:],
                                in0=ps[:],
                                scalar=1.0,
                                in1=w0_t[0:M, :],
                                op0=mybir.AluOpType.mult,
                                op1=mybir.AluOpType.mult,
                                accum_out=y_all[0:M, c:c + 1],
                            )
                        t0 += tsz
                    b0 += bsz
            nc.sync.dma_start(out_r[:, :], y_all[0:M, :])
    nc.finalize()
    return nc


def _prep(inputs):
    x = np.ascontiguousarray(np.asarray(inputs["scalar_representation"], dtype=np.float32))
    idx = np.asarray(inputs["idx_m"]).astype(np.int64)
    W = np.asarray(inputs["W"], dtype=np.float32)
    b = np.asarray(inputs["b"], dtype=np.float32)
    n = x.shape[0]
    A_max = NB * P

    mol_start = np.searchsorted(idx, np.arange(N_MOL + 1), side="left")

    targets = (np.arange(NCORES + 1) * n) // NCORES
    mcut = np.searchsorted(mol_start, targets, side="left").astype(np.int64)
    mcut[0], mcut[-1] = 0, N_MOL

    # Greedy chunking per core: up to M consecutive molecules per chunk,
    # atoms must fit in A_max rows (exact via searchsorted).
    core_chunks = []  # per core: list of (astart, aend, gm, nmols)
    for i in range(NCORES):
        chunks = []
        gm = int(mcut[i])
        gend = int(mcut[i + 1])
        while gm < gend:
            hi_atom_lim = int(np.searchsorted(
                mol_start, mol_start[gm] + A_max, side="right")) - 1
            hi = min(gm + M, gend, hi_atom_lim)
            nm = hi - gm
            assert nm > 0
            chunks.append((int(mol_start[gm]), int(mol_start[hi]), gm, nm))
            gm = hi
        core_chunks.append(chunks)
    NCH = max(len(ch) for ch in core_chunks)

    IOTA_OFF = 0
    LIDX_OFF = BTT * NB * M
    W0_OFF = LIDX_OFF + NCH * NB
    W0_OFF += W0_OFF % 2
    CW = W0_OFF + 2 * NFA
    iota_row = np.tile(np.arange(M, dtype=np.float32), BTT * NB).astype(
        ml_dtypes.bfloat16)
    w0aug_row = np.concatenate([W[0], b[0:1]]).astype(np.float32).view(ml_dtypes.bfloat16)

    in_maps = []
    for i in range(NCORES):
        chunks = core_chunks[i]
        xw_i = np.zeros((NCH * A_max, NFA), dtype=ml_dtypes.bfloat16)
        lidx_flat = np.full(NCH * A_max, -1.0, dtype=np.float32)
        for c, (astart, aend, gm, nm) in enumerate(chunks):
            spn = aend - astart
            if spn <= 0:
                continue
            xw_i[c * A_max:c * A_max + spn, 0:N_IN] = x[astart:aend]
            xw_i[c * A_max:c * A_max + spn, N_IN] = 1.0
            lidx_flat[c * A_max:c * A_max + spn] = idx[astart:aend] - gm
        lidx_t = lidx_flat.reshape(NCH, P, NB).transpose(1, 0, 2).reshape(
            P, NCH * NB).astype(ml_dtypes.bfloat16)
        parts = []
        for gstart, gc in _chunk_groups(NCH):
            blk = xw_i[gstart * A_max:(gstart + gc) * A_max]
            parts.append(np.ascontiguousarray(
                blk.reshape(gc, P, NB, NFA).transpose(1, 0, 2, 3)
                   .reshape(gc * A_max, NFA)))
        xw_i = np.concatenate(parts, axis=0)
        cst = np.zeros((P, CW), dtype=ml_dtypes.bfloat16)
        cst[:, IOTA_OFF:IOTA_OFF + BTT * NB * M] = iota_row[None, :]
        cst[:, LIDX_OFF:LIDX_OFF + NCH * NB] = lidx_t
        cst[:, W0_OFF:W0_OFF + 2 * NFA] = w0aug_row[None, :]
        in_maps.append({"xw": xw_i, "cst": np.ascontiguousarray(cst)})
    return in_maps, core_chunks, NCH


def _run(inputs, trace=False):
    from concourse import bass_utils

    in_maps, core_chunks, NCH = _prep(inputs)
    key = (NCH,)
    if key not in _graph_cache:
        _graph_cache[key] = _build_graph(NCH)
    nc = _graph_cache[key]

    res = bass_utils.run_bass_kernel_spmd(
        nc, in_maps, core_ids=list(range(NCORES)), trace=trace
    )
    y = np.zeros(N_MOL, dtype=np.float32)
    for i in range(NCORES):
        arr = res.results[i]["out"].reshape(M, NCH)
        for c, (astart, aend, gm, nm) in enumerate(core_chunks[i]):
            y[gm:gm + nm] = arr[0:nm, c]
    return y, res


def kernel(**inputs) -> np.ndarray:
    y, _ = _run(inputs, trace=False)
    return y
